# revision 71
# baseline (speedup 1.0000x reference)
"""Trainium2 Bass kernel for nn_MiniARDepthTransformer (forward_step, S=T=1).

Math notes (exact simplifications of the reference):
  - Attention with a single key/query position: softmax over an axis of
    size 1 is identically 1.0, so attention(x, ctx) == (ctx @ Wv) @ Wo.
    The q/k projections never affect the output.
  - Self-attn:  x += rmsnorm(x, sa_n) @ (diag(sa_n) sa_v @ sa_o)
    Cross-attn: x += context @ (ca_v @ ca_o)   (independent of x!)
  - Norm weights fold into the following matmul weights; done on host in f64.

Precision plan (from a host-side fp8-e4m3 noise study; gate is 2e-2):
  - FFN g/u matmuls and the last layer's down-proj run fp8 DoubleRow with
    same-scale (hi, lo) weight pairs: lo quantizes the residual of hi, both
    rows contract against a stride-0-broadcast fp8 activation in one DR
    matmul. Weight-side precision is ~bf16 at 2x bf16 PE throughput; the
    surviving noise is the plain-fp8 activation side (~1.85e-2 total).
  - Norm sum-of-squares also runs fp8 DR: squares are cast to fp8 at a
    per-site scale folded into ACT Square and compensated exactly in the
    Sqrt scale.
  - attn / ctx / unembed / early down-projs stay bf16 (fp8 there blows the
    error gate: attn and ctx noise is coherent across layers).

Schedule (stream-major, 2 batch streams of 512 per core):
  - Per layer: attn(b0) -> attn(b1) -> g/u(b0) -> down(b0) -> g/u(b1) ->
    down(b1), with each stream's rmsnorm chain hidden under the other
    stream's matmuls and the next layer's norm1 squares/ss interleaved into
    down(b1). ACT switches between the Sqrt and Silu table sets four times
    per layer, each load pinned (dummy activation) where ACT is idle.
  - The final unembed uses raw bf16 x as lhsT; rsqrt is applied per-sample
    via a per-partition ACT scale on the psum evac (binvT comes from tiny
    transposed ap_size=1 matmuls), so no xhat is materialized.
  - Weight DMAs issue on the sync queue in need order; the first ctx chunks
    split across SWDGE/HWDGE paths so the first matmul starts ~2.5us in.

Sharding: pure data parallel over batch, 8 cores x 1024 rows; weights
replicated; no collectives.
"""

import os
import sys

import numpy as np

try:
    import concourse.bass  # noqa: F401
except ImportError:
    for _p in (os.environ.get("TRN_RL_REPO"), "/opt/trn_rl_repo",
               "/root/.axon_site/_ro/trn_rl_repo"):
        if _p and os.path.isdir(_p):
            sys.path.insert(0, _p)
            break

import ml_dtypes
from concourse import bacc, mybir, tile
from concourse import bass_utils

P = 128
D = 512
F = 1024
V = 2048
MD = 768
L = 4
EPS = 1e-6
N_CORES = 8
B = 8192
BC = B // N_CORES        # 1024 rows per core
BS = 512                 # matmul free-dim chunk (one PSUM bank of fp32)
NB = BC // BS            # 2 batch streams
DK = D // P              # 4
FK = F // P              # 8
MK = MD // P             # 6
VK = V // BS             # 4
BT = BC // P             # 8 output row-tiles

BF16 = mybir.dt.bfloat16
F32 = mybir.dt.float32
FP8 = mybir.dt.float8e4
AF = mybir.ActivationFunctionType
DR = mybir.MatmulPerfMode.DoubleRow

WS = 64.0      # fp8 weight scale (hi/lo pairs share it)
XS = 8.0       # fp8 activation scale (xhat2)
GS = 32.0      # fp8 gu-tile scale (down-proj input, whilo layers)
DOWN8 = (False, False, False, True)   # per-layer down-proj fp8-DR hi/lo

_CACHE = {}


def _build_nc():
    nc = bacc.Bacc("TRN2", target_bir_lowering=False, debug=False,
                   num_devices=N_CORES)

    x0_d = nc.dram_tensor("x0", [P, DK, BC], BF16, kind="ExternalInput")
    mh_d = nc.dram_tensor("mh", [P, MK, BC], BF16, kind="ExternalInput")
    wc_d = nc.dram_tensor("wc", [P, MK, D], BF16, kind="ExternalInput")
    bc_d = nc.dram_tensor("bcv", [P, DK], F32, kind="ExternalInput")
    wsa_d = nc.dram_tensor("wsa", [L, P, DK, D], BF16, kind="ExternalInput")
    wca_d = nc.dram_tensor("wca", [L, P, DK, D], BF16, kind="ExternalInput")
    wg_d = nc.dram_tensor("wg", [L, P, DK, 2, F], FP8, kind="ExternalInput")
    wu_d = nc.dram_tensor("wu", [L, P, DK, 2, F], FP8, kind="ExternalInput")
    wd_d = nc.dram_tensor("wd", [L, P, FK, D], BF16, kind="ExternalInput")
    wdq_d = nc.dram_tensor("wdq", [L, P, FK, 2, D], FP8, kind="ExternalInput")
    wo_d = nc.dram_tensor("wo", [P, DK, V], BF16, kind="ExternalInput")
    out_d = nc.dram_tensor("out", [BC, V], F32, kind="ExternalOutput")

    with tile.TileContext(nc) as tc:
        with (
            tc.tile_pool(name="consts", bufs=1) as consts,
            tc.tile_pool(name="persist", bufs=1) as persist,
            tc.tile_pool(name="wpool", bufs=2) as wpool,
            tc.tile_pool(name="norm", bufs=2) as norm,
            tc.tile_pool(name="gup", bufs=2) as gup,
            tc.tile_pool(name="sgp", bufs=4) as sgp,
            tc.tile_pool(name="stage", bufs=6) as stage,
            tc.tile_pool(name="ps_mm", bufs=4, space="PSUM") as ps_mm,
            tc.tile_pool(name="ps_aux", bufs=4, space="PSUM") as ps_aux,
        ):
            ones = consts.tile([P, P], BF16)
            nc.vector.memset(ones, 1.0)
            ones8 = consts.tile([P, 2, P], FP8)
            nc.vector.memset(ones8, 1.0)
            eps_t = consts.tile([P, 1], F32)
            nc.vector.memset(eps_t, EPS)
            eps_t8 = consts.tile([P, 1], F32)
            nc.vector.memset(eps_t8, EPS / (XS * XS))
            dmy_in = consts.tile([P, 1], F32)
            nc.vector.memset(dmy_in, 1.0)
            dmy_out = consts.tile([P, 1], F32)

            def preload_act(func, dep_ap=None):
                # Tiny activation pinning this table-set's load where the ACT
                # engine is idle instead of on the critical path.
                nc.scalar.activation(out=dmy_out[:],
                                     in_=dep_ap if dep_ap is not None
                                     else dmy_in[:],
                                     func=func, bias=eps_t[:], scale=0.0)

            def bsl(b):
                return slice(b * BS, (b + 1) * BS)

            def psl(n):
                return slice(n * P, (n + 1) * P)

            def bcast2(ap):
                return ap.unsqueeze(1).broadcast_to(
                    (ap.shape[0], 2, ap.shape[1]))

            # ---- DMA schedule: what the first matmuls need comes first ----
            wc = consts.tile([P, MK, D], BF16)
            bcT = consts.tile([P, DK], F32)
            mh = persist.tile([P, MK, BC], BF16)
            x = persist.tile([P, DK, BC], F32)
            xbf = persist.tile([P, DK, BC], BF16)
            ctx = persist.tile([P, DK, BC], BF16)
            wout = consts.tile([P, DK, V], BF16)

            # first k-chunk pair goes out on two independent DGE paths so
            # their descriptor pipelines overlap (SWDGE via Pool, HWDGE via
            # sync) and the first ctx matmul starts ~1.3us earlier
            nc.gpsimd.dma_start(out=wc[:, 0:1], in_=wc_d.ap()[:, 0:1])
            nc.sync.dma_start(out=mh[:, 0:1, bsl(0)],
                              in_=mh_d.ap()[:, 0:1, bsl(0)])
            nc.scalar.dma_start(out=bcT[:], in_=bc_d.ap())
            nc.gpsimd.dma_start(out=wc[:, 1:2], in_=wc_d.ap()[:, 1:2])
            nc.sync.dma_start(out=mh[:, 1:2, bsl(0)],
                              in_=mh_d.ap()[:, 1:2, bsl(0)])
            # x0 lands in the xbf tile (bf16): layer 0 reads it as the
            # residual base; the tile is reused for the final bf16 x.
            nc.scalar.dma_start(out=xbf[:, :, bsl(0)],
                                in_=x0_d.ap()[:, :, bsl(0)])
            for ks in (slice(2, 4), slice(4, 6)):
                nc.gpsimd.dma_start(out=wc[:, ks], in_=wc_d.ap()[:, ks])
                nc.sync.dma_start(out=mh[:, ks, bsl(0)],
                                  in_=mh_d.ap()[:, ks, bsl(0)])
            for ks in (slice(0, 3), slice(3, 6)):
                nc.sync.dma_start(out=mh[:, ks, bsl(1)],
                                  in_=mh_d.ap()[:, ks, bsl(1)])
            wca = [wpool.tile([P, DK, D], BF16, tag="wca",
                              name=f"wca{_j}") for _j in range(2)]
            wsa = [wpool.tile([P, DK, D], BF16, tag="wsa",
                              name=f"wsa{_j}") for _j in range(2)]
            nc.sync.dma_start(out=wca[0][:], in_=wca_d.ap()[0])
            nc.sync.dma_start(out=xbf[:, :, bsl(1)],
                              in_=x0_d.ap()[:, :, bsl(1)])
            nc.sync.dma_start(out=wsa[0][:], in_=wsa_d.ap()[0])
            # Sqrt table-set load overlaps the initial DMAs.
            preload_act(AF.Sqrt)

            def rmsnorm(b, fp8=False, tag="n1", src_t=None, sqs=8.0):
                """xhat = x*rsqrt(mean(x^2)+eps) for stream b. Squares go to
                fp8 at per-site scale sqs (folded into the Square input,
                compensated exactly in the Sqrt scale) so the sum-of-squares
                matmuls run fp8-DoubleRow."""
                bs = bsl(b)
                sq = norm.tile([P, DK, BS], FP8, tag="sq8")
                xhat = norm.tile([P, DK, BS], FP8 if fp8 else BF16,
                                 tag="xh" + tag)

                xs = x if src_t is None else src_t
                sroot = float(np.sqrt(sqs))

                def emit_sq_ss():
                    for d in range(DK):
                        nc.scalar.activation(out=sq[:, d], in_=xs[:, d, bs],
                                             func=AF.Square, scale=sroot)
                    ss = ps_mm.tile([P, BS], F32, tag="mm", name="ss")
                    for h in range(2):
                        nc.tensor.matmul(ss[:], ones8[:, :, :],
                                         sq[:, 2 * h:2 * h + 2],
                                         start=(h == 0), stop=(h == 1),
                                         perf_mode=DR)
                    return ss

                srt = norm.tile([P, BS], F32, tag="srt", bufs=3)

                def emit_tail(ss):
                    # srt = sqrt(ms+eps)/XS so binv = XS * rsqrt(ms+eps)
                    if fp8:
                        nc.scalar.activation(out=srt[:], in_=ss[:],
                                             func=AF.Sqrt, bias=eps_t8[:],
                                             scale=1.0 / (D * sqs * XS * XS))
                    else:
                        nc.scalar.activation(out=srt[:], in_=ss[:],
                                             func=AF.Sqrt, bias=eps_t[:],
                                             scale=1.0 / (D * sqs))
                    nc.vector.reciprocal_approx_fast(out=srt[:], in_=srt[:])
                    # split multiply across DVE + Pool (SBUF-only: Pool
                    # cannot touch PSUM). The fp8 norm feeds DR matmuls that
                    # consume k-chunks in order, so give DVE the leading
                    # chunks (shorter chain to the first consumer) and the
                    # slower Pool engine only the tail chunk.
                    bb = srt[:].unsqueeze(1).broadcast_to((P, 2, BS))
                    nc.vector.tensor_mul(out=xhat[:, 0:2, :],
                                         in0=xs[:, 0:2, bs], in1=bb)
                    nc.gpsimd.tensor_mul(out=xhat[:, 2:4, :],
                                         in0=xs[:, 2:4, bs], in1=bb)
                return xhat, emit_sq_ss, emit_tail, srt

            # ---- context = mh @ Wc + bc (bf16); layer-0's norm1 for each
            # stream is emitted right behind that stream's ctx so the ACT
            # queue order matches data-arrival order ----
            _l0n1 = [None, None]

            def emit_ctx_n1(b):
                pss = []
                for n in range(DK):
                    ps = ps_mm.tile([P, BS], F32, tag="mm", name="ctxps")
                    pss.append(ps)
                for k in range(MK):
                    for n in range(DK):
                        nc.tensor.matmul(pss[n][:], wc[:, k, psl(n)],
                                         mh[:, k, bsl(b)],
                                         start=(k == 0), stop=(k == MK - 1))
                _l0n1[b] = rmsnorm(b, tag="n1", src_t=xbf, sqs=5000.0)
                ss1 = None
                if b == 0:
                    # b0: x0 arrives before the ctx psums drain, so the norm
                    # squares go ahead of the evacs in the ACT queue
                    ss1 = _l0n1[b][1]()
                for n in range(DK):
                    nc.scalar.activation(out=ctx[:, n, bsl(b)], in_=pss[n][:],
                                         func=AF.Identity,
                                         bias=bcT[:, n:n + 1], scale=1.0)
                if b != 0:
                    ss1 = _l0n1[b][1]()
                _l0n1[b][2](ss1)

            emit_ctx_n1(0)
            emit_ctx_n1(1)


            for i in range(L):
                nxt = (i + 1) % 2
                cur = i % 2
                wg = wpool.tile([P, DK, 2, F], FP8, tag="wg")
                nc.sync.dma_start(out=wg[:], in_=wg_d.ap()[i])
                wu = wpool.tile([P, DK, 2, F], FP8, tag="wu")
                nc.sync.dma_start(out=wu[:], in_=wu_d.ap()[i])
                if DOWN8[i]:
                    wd = wpool.tile([P, FK, 2, D], FP8, tag="wdq", bufs=1)
                    nc.sync.dma_start(out=wd[:], in_=wdq_d.ap()[i])
                else:
                    wd = wpool.tile([P, FK, D], BF16, tag="wd", bufs=1)
                    nc.sync.dma_start(out=wd[:], in_=wd_d.ap()[i])
                if i < L - 1:
                    nc.sync.dma_start(out=wca[nxt][:], in_=wca_d.ap()[i + 1])
                    nc.sync.dma_start(out=wsa[nxt][:], in_=wsa_d.ap()[i + 1])
                if i == L - 1:
                    nc.sync.dma_start(out=wout[:], in_=wo_d.ap())

                # ---- norm1 tails (sq/ss were emitted during the
                # previous layer's down-proj; for L0 emitted here) ----
                if i == 0:
                    n1 = _l0n1
                else:
                    n1 = _pending_n1
                    for b in range(NB):
                        n1[b][2](_pending_ss[b])
                def xhat_of(b):
                    return n1[b][0]

                # ---- stream-major attn: finish b0 entirely so its FFN norm
                # chain hides under b1's attn matmuls ----
                n2 = [None, None]
                ss2 = [None, None]
                for b in range(NB):
                    pss = []
                    for n in range(DK):
                        ps = ps_mm.tile([P, BS], F32, tag="mm")
                        pss.append(ps)
                        for k in range(DK):
                            nc.tensor.matmul(ps[:], wca[cur][:, k, psl(n)],
                                             ctx[:, k, bsl(b)],
                                             start=(k == 0), stop=False)
                    res_in = xbf if i == 0 else x
                    for n in range(DK):
                        for k in range(DK):
                            nc.tensor.matmul(pss[n][:],
                                             wsa[cur][:, k, psl(n)],
                                             xhat_of(b)[:, k],
                                             start=False, stop=(k == DK - 1))
                        nc.vector.tensor_add(out=x[:, n, bsl(b)],
                                             in0=res_in[:, n, bsl(b)],
                                             in1=pss[n][:])
                    # norm2(b): squares+ss now; tail right away for b0 so
                    # xhat2(b0) computes during b1's attn.
                    n2[b] = rmsnorm(b, fp8=True, tag="n2")
                    ss2[b] = n2[b][1]()
                    if b == 0:
                        n2[0][2](ss2[0])

                preload_act(AF.Silu, n2[0][3][:, 0:1])

                # ---- SwiGLU FFN, stream-interleaved: g/u(b0), down(b0),
                # g/u(b1), down(b1). Four ACT table loads per layer, each
                # pinned where ACT is otherwise idle, so neither the silu
                # nor the next sqrt load ever gates PE. ----
                last_layer = (i == L - 1)
                dsc = 1.0 / (WS * GS) if DOWN8[i] else 1.0
                gus = [None, None]
                sgl = [None, None]

                def emit_gu(b):
                    xh2 = n2[b][0]
                    gu = gup.tile([P, FK, BS], FP8 if DOWN8[i] else BF16,
                                  tag="guq" if DOWN8[i] else "gu", name="gu")
                    gus[b] = gu
                    for f in range(FK):
                        psg = ps_aux.tile([P, BS], F32, tag="aux", name="psg")
                        psu = ps_aux.tile([P, BS], F32, tag="aux", name="psu")
                        for k in range(DK):
                            rhs = bcast2(xh2[:, k, :])
                            nc.tensor.matmul(psg[:], wg[:, k, :, psl(f)], rhs,
                                             start=(k == 0),
                                             stop=(k == DK - 1), perf_mode=DR)
                        for k in range(DK):
                            rhs = bcast2(xh2[:, k, :])
                            nc.tensor.matmul(psu[:], wu[:, k, :, psl(f)], rhs,
                                             start=(k == 0),
                                             stop=(k == DK - 1), perf_mode=DR)
                        sg = sgp.tile([P, BS], BF16, tag="sg")
                        sgl[b] = sg
                        nc.scalar.activation(out=sg[:], in_=psg[:],
                                             func=AF.Silu,
                                             scale=1.0 / (WS * XS))
                        nc.vector.scalar_tensor_tensor(
                            out=gu[:, f], in0=psu[:],
                            scalar=(GS if DOWN8[i] else 1.0) / (WS * XS),
                            in1=sg[:], op0=mybir.AluOpType.mult,
                            op1=mybir.AluOpType.mult)

                def emit_down(b):
                    for n in range(DK):
                        ps = ps_mm.tile([P, BS], F32, tag="mm")
                        if DOWN8[i]:
                            for k in range(FK):
                                nc.tensor.matmul(ps[:], wd[:, k, :, psl(n)],
                                                 bcast2(gus[b][:, k]),
                                                 start=(k == 0),
                                                 stop=(k == FK - 1),
                                                 perf_mode=DR)
                        else:
                            for k in range(FK):
                                nc.tensor.matmul(ps[:], wd[:, k, psl(n)],
                                                 gus[b][:, k], start=(k == 0),
                                                 stop=(k == FK - 1))
                        if last_layer:
                            nc.vector.scalar_tensor_tensor(
                                out=xbf[:, n, bsl(b)], in0=ps[:], scalar=dsc,
                                in1=x[:, n, bsl(b)],
                                op0=mybir.AluOpType.mult,
                                op1=mybir.AluOpType.add)
                        elif DOWN8[i]:
                            nc.vector.scalar_tensor_tensor(
                                out=x[:, n, bsl(b)], in0=ps[:], scalar=dsc,
                                in1=x[:, n, bsl(b)],
                                op0=mybir.AluOpType.mult,
                                op1=mybir.AluOpType.add)
                        else:
                            nc.vector.tensor_add(out=x[:, n, bsl(b)],
                                                 in0=x[:, n, bsl(b)],
                                                 in1=ps[:])
                        if b == 1 and not last_layer and n == 1:
                            _pending_ss[0] = _pending_n1[0][1]()

                emit_gu(0)
                preload_act(AF.Sqrt, sgl[0][:, 0:1])
                n2[1][2](ss2[1])              # sqrt2(b1) mid-down(b0)
                preload_act(AF.Silu, n2[1][3][:, 0:1])
                emit_down(0)
                emit_gu(1)
                preload_act(AF.Sqrt, sgl[1][:, 0:1])
                if not last_layer:
                    _pending_n1 = [rmsnorm(bb, tag="n1") for bb in range(NB)]
                    _pending_ss = [None, None]
                emit_down(1)
                if not last_layer:
                    _pending_ss[1] = _pending_n1[1][1]()

            # ---- final norm + unembed ----
            # Raw bf16 x is the lhsT; rsqrt applied per-sample via a
            # per-partition ACT scale on the psum evac. ssT[sample] comes
            # from tiny transposed matmuls (ap_size=1).
            sqf = [norm.tile([P, DK, BS], BF16, tag="sq",
                             name=f"sqf{_j}") for _j in range(NB)]
            binvT = consts.tile([P, BT], F32)
            for b in range(NB):
                bs = bsl(b)
                for d in range(DK):
                    nc.scalar.activation(out=sqf[b][:, d], in_=xbf[:, d, bs],
                                         func=AF.Square)
            nbt = BT // NB
            for b in range(NB):
                ssT = ps_aux.tile([P, BS], F32, tag="aux")
                for j in range(nbt):
                    for d in range(DK):
                        nc.tensor.matmul(ssT[:, j:j + 1],
                                         sqf[b][:, d, psl(j)], ones[:, 0:1],
                                         start=(d == 0), stop=(d == DK - 1))
                srtT = norm.tile([P, BT], F32, tag="srtT")
                nc.scalar.activation(out=srtT[:, 0:nbt], in_=ssT[:, 0:nbt],
                                     func=AF.Sqrt,
                                     bias=eps_t[:], scale=1.0 / D)
                nc.vector.reciprocal_approx_fast(
                    out=binvT[:, b * nbt:(b + 1) * nbt], in_=srtT[:, 0:nbt])

            _uc = 0
            for b in range(NB):
                for bt in range(BT // NB):
                    j = b * (BT // NB) + bt
                    for v in range(VK):
                        pool = ps_mm if _uc % 2 == 0 else ps_aux
                        _uc += 1
                        ps = pool.tile([P, BS], F32,
                                       tag="mm" if pool is ps_mm else "aux")
                        for k in range(DK):
                            nc.tensor.matmul(ps[:], xbf[:, k, psl(j)],
                                             wout[:, k, bsl(v)],
                                             start=(k == 0),
                                             stop=(k == DK - 1))
                        st = stage.tile([P, BS], F32, tag="st")
                        nc.scalar.activation(out=st[:], in_=ps[:],
                                             func=AF.Identity,
                                             scale=binvT[:, j:j + 1])
                        deng = nc.sync if (_uc % 2 == 0) else nc.scalar
                        deng.dma_start(out=out_d.ap()[psl(j), bsl(v)],
                                       in_=st[:])

    nc.finalize()
    return nc


def _get_nc():
    if "nc" not in _CACHE:
        _CACHE["nc"] = _build_nc()
    return _CACHE["nc"]


def _bf16(a):
    return np.ascontiguousarray(a).astype(ml_dtypes.bfloat16)


NPFP8 = mybir.dt.np(mybir.dt.float8e4)


def _kxn(w):
    # [K, N] -> [P, K//P, N]: [:, k, n0:n0+128] is a natural lhsT chunk.
    K, N = w.shape
    return np.ascontiguousarray(w.reshape(K // P, P, N).transpose(1, 0, 2))


def _pair_kxn(w, scale):
    # [K, N] f64 -> fp8 (hi, lo) pairs [P, K//P, 2, N] at shared scale.
    K, N = w.shape
    ws = (w * scale).astype(np.float32)
    hi = ws.astype(NPFP8)
    lo = (ws - hi.astype(np.float32)).astype(NPFP8)
    st = np.stack([hi, lo], axis=1)            # [K, 2, N]
    st = st.reshape(K // P, P, 2, N).transpose(1, 0, 2, 3)
    assert np.isfinite(st.astype(np.float32)).all()
    return np.ascontiguousarray(st)


def _prep_inputs(inputs):
    f = {k: np.asarray(v) for k, v in inputs.items()}
    prev = f["prev_tokens"].reshape(-1).astype(np.int64)
    emb = f["emb"].astype(np.float32)
    mhf = f["main_hidden"].reshape(B, MD).astype(np.float32)

    x0 = emb[prev]                                  # [B, D] f32
    x0c = x0.reshape(N_CORES, BC, DK, P).transpose(0, 3, 2, 1)
    x0c = [_bf16(x0c[c]) for c in range(N_CORES)]
    mhc = mhf.reshape(N_CORES, BC, MK, P).transpose(0, 3, 2, 1)
    mhc = [_bf16(mhc[c]) for c in range(N_CORES)]

    f64 = lambda k: f[k].astype(np.float64)
    sa_n, sa_v, sa_o = f64("sa_n"), f64("sa_v"), f64("sa_o")
    ca_v, ca_o = f64("ca_v"), f64("ca_o")
    ffn_n, w_g, w_u, w_d = f64("ffn_n"), f64("w_g"), f64("w_u"), f64("w_d")

    wsa = np.stack([_kxn((sa_n[i][:, None] * sa_v[i]) @ sa_o[i])
                    for i in range(L)])
    wca = np.stack([_kxn(ca_v[i] @ ca_o[i]) for i in range(L)])
    wg = np.stack([_pair_kxn(ffn_n[i][:, None] * w_g[i], WS)
                   for i in range(L)])
    wu = np.stack([_pair_kxn(ffn_n[i][:, None] * w_u[i], WS)
                   for i in range(L)])
    wd = np.stack([_kxn(w_d[i]) for i in range(L)])
    wdq = np.stack([_pair_kxn(w_d[i], WS) for i in range(L)])
    wo = _kxn(f64("final_n")[:, None] * f64("Wout"))
    wcp = _bf16(_kxn(f64("Wc")))
    bcv = np.ascontiguousarray(
        f["bc"].astype(np.float32).reshape(DK, P).T)

    shared = {
        "wc": wcp, "bcv": bcv.astype(np.float32),
        "wsa": _bf16(wsa), "wca": _bf16(wca),
        "wg": wg, "wu": wu, "wd": _bf16(wd), "wdq": wdq,
        "wo": _bf16(wo),
    }
    in_maps = []
    for c in range(N_CORES):
        m = dict(shared)
        m["x0"] = x0c[c]
        m["mh"] = mhc[c]
        in_maps.append(m)
    return in_maps


def _run(inputs, trace=False, **kw):
    nc = _get_nc()
    in_maps = _prep_inputs(inputs)
    res = bass_utils.run_bass_kernel_spmd(
        nc, in_maps, core_ids=list(range(N_CORES)), trace=trace, **kw)
    out = np.concatenate([res.results[c]["out"] for c in range(N_CORES)],
                         axis=0)
    return out, res


def kernel(**inputs) -> np.ndarray:
    out, _ = _run(inputs, trace=False)
    return out
